# revision 2
# baseline (speedup 1.0000x reference)
"""Trainium2 Bass kernel for LocalRelationDistillLoss.

Full inputs: student_emb [16,1024,768] f32, teacher_emb [16,1024,768] f32,
centers [16,1024,2] f32. Output: scalar f32 loss.

Strategy: data-parallel over batch across 8 NeuronCores (2 batch elements per
core). Per batch element:
  - v = -pairwise_sq_dist(centers) [1024,1024] via a rank-4 augmented matmul
    (factors built on host from centers), with -1e30 added on the diagonal
    (extra identity matmul) so self-matches are excluded.
  - per-row kNN threshold t = 8th-largest of v via the DVE max8 instruction;
    mask = (v >= t) selects exactly the 8 nearest neighbors.
  - cosine similarities via Gram matmul of row-normalized embeddings (bf16).
    Student Gram and negated-teacher Gram accumulate into the same PSUM so
    d = cos_s - cos_t comes out of the PE directly.
  - dm = mask * d in one fused DVE op; smooth-L1(beta=0.5) via
    sl1 = dm^2 - relu(|dm| - 0.5)^2, accumulated with free-dim accum_out.
Per-core output: [128,1] partial sums; host sums and divides.
"""

import os

os.environ.setdefault("MYCRO_LOCAL_CACHE", "1")

import numpy as np

import concourse.bass as bass
import concourse.tile as tile
from concourse import bacc, mybir
from concourse import bass_utils

F32 = mybir.dt.float32
BF16 = mybir.dt.bfloat16

B = 16
N = 1024
D = 768
NCORES = 8
BPC = B // NCORES          # batch elements per core
NRB = N // 128             # row blocks
KC = D // 128              # contraction chunks
BETA = 0.5
EPS = 1e-8
NEG_BIG = -1.0e30

_cache = {}


def _build_nc():
    nc = bacc.Bacc("TRN2", target_bir_lowering=False, debug=False)

    student = nc.dram_tensor("student", [BPC, N, D], F32, kind="ExternalInput")
    teacher = nc.dram_tensor("teacher", [BPC, N, D], F32, kind="ExternalInput")
    # augmented rank-4 factors for v = -d2: af is lhsT [4, BPC*N], bf is rhs
    af = nc.dram_tensor("af", [4, BPC * N], F32, kind="ExternalInput")
    bf = nc.dram_tensor("bf", [4, BPC * N], F32, kind="ExternalInput")
    out = nc.dram_tensor("out", [128, 1], F32, kind="ExternalOutput")

    eye_h = nc.inline_tensor(np.eye(128, dtype=np.float32), "eye128")
    dneg_h = nc.inline_tensor(
        (NEG_BIG * np.eye(128)).astype(np.float32), "diagneg"
    )

    AF = mybir.ActivationFunctionType
    OP = mybir.AluOpType

    with tile.TileContext(nc) as tc:
        with (
            tc.tile_pool(name="const", bufs=1) as cpool,
            tc.tile_pool(name="raw", bufs=2) as rawp,
            tc.tile_pool(name="ehat", bufs=4) as ehatp,
            tc.tile_pool(name="sqscr", bufs=2) as sqscrp,
            tc.tile_pool(name="nrm", bufs=4) as nrmp,
            tc.tile_pool(name="eT", bufs=1) as eTp,
            tc.tile_pool(name="vsb", bufs=2) as vsbp,
            tc.tile_pool(name="small", bufs=4) as smallp,
            tc.tile_pool(name="accs", bufs=1) as accp,
            tc.tile_pool(name="dram", bufs=2, space="DRAM") as dramp,
            tc.tile_pool(name="psv", bufs=2, space="PSUM") as psvp,
            tc.tile_pool(name="psd", bufs=2, space="PSUM") as psdp,
        ):
            eye_sb = cpool.tile([128, 128], F32, tag="eye")
            dneg_sb = cpool.tile([128, 128], F32, tag="dneg")
            nc.sync.dma_start(out=eye_sb[:], in_=eye_h.ap())
            nc.sync.dma_start(out=dneg_sb[:], in_=dneg_h.ap())
            af_sb = cpool.tile([4, BPC * N], F32, tag="af")
            bf_sb = cpool.tile([4, BPC * N], F32, tag="bf")
            nc.sync.dma_start(out=af_sb[:], in_=af.ap())
            nc.sync.dma_start(out=bf_sb[:], in_=bf.ap())

            acc_d2 = accp.tile([128, BPC * NRB], F32, tag="acc_d2")
            acc_u2 = accp.tile([128, BPC * NRB], F32, tag="acc_u2")

            for b in range(BPC):
                # ---- normalize + transpose both embedding matrices ----
                eTs = eTp.tile([128, KC, N], BF16, tag=f"eTs{b % 2}")
                eTt = eTp.tile([128, KC, N], BF16, tag=f"eTt{b % 2}")
                eTtn = eTp.tile([128, KC, N], BF16, tag=f"eTtn{b % 2}")
                for src, dst in ((student, eTs), (teacher, eTt)):
                    raw = rawp.tile([128, NRB, D], F32, tag="raw")
                    nc.sync.dma_start(
                        out=raw[:],
                        in_=src.ap()[b].rearrange("(r p) d -> p r d", p=128),
                    )
                    nrm2 = nrmp.tile([128, NRB], F32, tag="nrm2")
                    for rb in range(NRB):
                        sqs = sqscrp.tile([128, D], F32, tag="sqs")
                        nc.scalar.activation(
                            sqs[:], raw[:, rb], AF.Square,
                            accum_out=nrm2[:, rb : rb + 1],
                        )
                    rinv = nrmp.tile([128, NRB], F32, tag="rinv")
                    nc.scalar.activation(rinv[:], nrm2[:], AF.Sqrt)
                    nc.vector.tensor_scalar_max(rinv[:], rinv[:], EPS)
                    nc.vector.reciprocal(rinv[:], rinv[:])
                    stage = dramp.tile([N, D], BF16, tag="stage")
                    for rb in range(NRB):
                        eh = ehatp.tile([128, D], BF16, tag="ehat")
                        nc.scalar.activation(
                            eh[:], raw[:, rb], AF.Copy,
                            scale=rinv[:, rb : rb + 1],
                        )
                        nc.sync.dma_start(
                            out=stage[rb * 128 : (rb + 1) * 128, :], in_=eh[:]
                        )
                    for c in range(KC):
                        nc.sync.dma_start(
                            out=dst[:, c, :],
                            in_=stage[:, c * 128 : (c + 1) * 128],
                            transpose=True,
                        )
                for c in range(KC):
                    nc.vector.tensor_scalar_mul(eTtn[:, c, :], eTt[:, c, :], -1.0)

                # ---- per row-block: knn threshold + Gram-diff + loss ----
                for rb in range(NRB):
                    rbs = slice(rb * 128, (rb + 1) * 128)
                    # v = -d2 with -inf diagonal
                    psv = psvp.tile([128, N], F32, tag="psv")
                    for half in range(2):
                        js = slice(half * 512, (half + 1) * 512)
                        diag_here = (rb // 4) == half
                        nc.tensor.matmul(
                            psv[:, js],
                            af_sb[:, b * N + rb * 128 : b * N + (rb + 1) * 128],
                            bf_sb[:, b * N + half * 512 : b * N + (half + 1) * 512],
                            start=True,
                            stop=True,
                        )
                        if diag_here:
                            nc.tensor.matmul(
                                psv[:, rbs],
                                eye_sb[:],
                                dneg_sb[:],
                                start=False,
                                stop=True,
                                skip_group_check=True,
                            )
                    vsb = vsbp.tile([128, N], F32, tag="vsb")
                    nc.scalar.activation(vsb[:], psv[:], AF.Copy)
                    vals8 = smallp.tile([128, 8], F32, tag="vals8")
                    nc.vector.max(vals8[:], vsb[:])

                    # d = cos_s - cos_t accumulated in PSUM
                    psd = psdp.tile([128, N], F32, tag="psd")
                    for half in range(2):
                        js = slice(half * 512, (half + 1) * 512)
                        for c in range(KC):
                            nc.tensor.matmul(
                                psd[:, js], eTs[:, c, rbs], eTs[:, c, js],
                                start=(c == 0), stop=False,
                            )
                        for c in range(KC):
                            nc.tensor.matmul(
                                psd[:, js], eTtn[:, c, rbs], eTt[:, c, js],
                                start=False, stop=(c == KC - 1),
                            )

                    # dm = (v >= t) * d   (t = 8th largest v in the row)
                    dm = smallp.tile([128, N], BF16, tag="dm")
                    nc.vector.scalar_tensor_tensor(
                        dm[:], vsb[:], vals8[:, 7:8], psd[:],
                        op0=OP.is_ge, op1=OP.mult,
                    )
                    # u = relu(|dm| - beta/2) = relu(dm-0.25*2)... (beta=0.5)
                    u1 = smallp.tile([128, N], BF16, tag="u1")
                    nc.vector.tensor_scalar(
                        u1[:], dm[:], 0.5 * BETA, 0.0, op0=OP.subtract, op1=OP.max
                    )
                    m2 = smallp.tile([128, N], BF16, tag="m2")
                    nc.vector.tensor_scalar(
                        m2[:], dm[:], 0.5 * BETA, 0.0, op0=OP.add, op1=OP.min
                    )
                    u = smallp.tile([128, N], BF16, tag="u")
                    nc.vector.tensor_sub(u[:], u1[:], m2[:])
                    # sum dm^2 and u^2 (sl1 = dm^2 - u^2 summed later)
                    col = b * NRB + rb
                    dsq = smallp.tile([128, N], BF16, tag="dsq")
                    nc.scalar.activation(
                        dsq[:], dm[:], AF.Square,
                        accum_out=acc_d2[:, col : col + 1],
                    )
                    usq = smallp.tile([128, N], BF16, tag="usq")
                    nc.vector.scalar_tensor_tensor(
                        usq[:], u[:], 1.0, u[:],
                        op0=OP.mult, op1=OP.mult,
                        accum_out=acc_u2[:, col : col + 1],
                    )

            s1 = smallp.tile([128, 1], F32, tag="s1")
            s2 = smallp.tile([128, 1], F32, tag="s2")
            nc.vector.reduce_sum(s1[:], acc_d2[:], axis=mybir.AxisListType.X)
            nc.vector.reduce_sum(s2[:], acc_u2[:], axis=mybir.AxisListType.X)
            osb = smallp.tile([128, 1], F32, tag="osb")
            nc.vector.tensor_sub(osb[:], s1[:], s2[:])
            nc.sync.dma_start(out=out.ap(), in_=osb[:])

    nc.compile()
    return nc


def _host_factors(centers_core: np.ndarray) -> tuple[np.ndarray, np.ndarray]:
    """Rank-4 factors so that af[:, i].T @ bf[:, j] = -||c_i - c_j||^2
    (within one batch element)."""
    x = centers_core[:, :, 0].astype(np.float32)  # [BPC, N]
    y = centers_core[:, :, 1].astype(np.float32)
    sq = x * x + y * y
    ones = np.ones_like(x)
    af = np.stack([-sq, -ones, 2.0 * x, 2.0 * y], axis=1)  # [BPC, 4, N]
    bf = np.stack([ones, sq, x, y], axis=1)                # [BPC, 4, N]
    af = np.ascontiguousarray(af.transpose(1, 0, 2).reshape(4, BPC * N))
    bf = np.ascontiguousarray(bf.transpose(1, 0, 2).reshape(4, BPC * N))
    return af, bf


def kernel(student_emb, teacher_emb, centers):
    student_emb = np.asarray(student_emb, dtype=np.float32)
    teacher_emb = np.asarray(teacher_emb, dtype=np.float32)
    centers = np.asarray(centers, dtype=np.float32)

    if "nc" not in _cache:
        _cache["nc"] = _build_nc()
    nc = _cache["nc"]

    in_maps = []
    for c in range(NCORES):
        lo, hi = c * BPC, (c + 1) * BPC
        af, bf = _host_factors(centers[lo:hi])
        in_maps.append(
            {
                "student": np.ascontiguousarray(student_emb[lo:hi]),
                "teacher": np.ascontiguousarray(teacher_emb[lo:hi]),
                "af": af,
                "bf": bf,
            }
        )

    res = bass_utils.run_bass_kernel_spmd(nc, in_maps, core_ids=list(range(NCORES)))
    total = np.float64(0.0)
    for c in range(NCORES):
        total += np.sum(res.results[c]["out"].astype(np.float64))
    loss = total / float(B * N * 8)
    return np.float32(loss)


# revision 14
# speedup vs baseline: 1.1699x; 1.1699x over previous
"""Trainium2 Bass kernel for LocalRelationDistillLoss.

Full inputs: student_emb [16,1024,768] f32, teacher_emb [16,1024,768] f32,
centers [16,1024,2] f32. Output: scalar f32 loss.

Strategy: data-parallel over batch across 8 NeuronCores (2 batch elements per
core). Per batch element:
  - v = -pairwise_sq_dist(centers) [1024,1024] via a rank-4 augmented matmul
    (factors built on host from centers), with -1e30 added on the diagonal
    (extra identity matmul) so self-matches are excluded.
  - per-row kNN threshold t = 8th-largest of v via the DVE max8 instruction;
    mask = (v >= t) selects exactly the 8 nearest neighbors.
  - cosine similarities via Gram matmul of row-normalized embeddings (bf16).
    Student Gram and negated-teacher Gram accumulate into the same PSUM so
    d = cos_s - cos_t comes out of the PE directly.
  - dm = mask * d in one fused DVE op; smooth-L1(beta=0.5) via
    sl1 = dm^2 - relu(|dm| - 0.5)^2, accumulated with free-dim accum_out.
Per-core output: [128,1] partial sums; host sums and divides.
"""

import os

os.environ.setdefault("MYCRO_LOCAL_CACHE", "1")

import numpy as np

import concourse.bass as bass
import concourse.tile as tile
from concourse import bacc, mybir
from concourse import bass_utils

F32 = mybir.dt.float32
F16 = mybir.dt.float16
BF16 = mybir.dt.bfloat16

B = 16
N = 1024
D = 768
NCORES = 8
BPC = B // NCORES          # batch elements per core
NRB = N // 128             # row blocks
KC = D // 128              # contraction chunks
BETA = 0.5
EPS = 1e-8
NEG_BIG = -1.0e30

_cache = {}


def _build_nc(opts=()):
    opts = set(opts)
    nc = bacc.Bacc("TRN2", target_bir_lowering=False, debug=False)

    student = nc.dram_tensor("student", [BPC, N, D], F32, kind="ExternalInput")
    teacher = nc.dram_tensor("teacher", [BPC, N, D], F32, kind="ExternalInput")
    # augmented fp16 split-precision factors for v = -d2 (hi*hi + hi*lo + lo*hi)
    af = nc.dram_tensor("af", [12, BPC * N], F16, kind="ExternalInput")
    bf = nc.dram_tensor("bf", [12, BPC * N], F16, kind="ExternalInput")
    out = nc.dram_tensor("out", [128, 1], F32, kind="ExternalOutput")

    eye_h = nc.inline_tensor(np.eye(128, dtype=np.float16), "eye128")
    dneg_h = nc.inline_tensor(
        (-60000.0 * np.eye(128)).astype(np.float16), "diagneg"
    )

    AF = mybir.ActivationFunctionType
    OP = mybir.AluOpType

    with tile.TileContext(nc) as tc:
        with (
            tc.tile_pool(name="const", bufs=1) as cpool,
            tc.tile_pool(name="raw", bufs=2) as rawp,
            tc.tile_pool(name="ehat", bufs=4) as ehatp,
            tc.tile_pool(name="sqscr", bufs=2) as sqscrp,
            tc.tile_pool(name="nrm", bufs=4) as nrmp,
            tc.tile_pool(name="eT", bufs=1) as eTp,
            tc.tile_pool(name="vsb", bufs=2) as vsbp,
            tc.tile_pool(name="small", bufs=4) as smallp,
            tc.tile_pool(name="accs", bufs=1) as accp,
            tc.tile_pool(name="dram", bufs=2, space="DRAM") as dramp,
            tc.tile_pool(
                name="psv", bufs=(1 if 'psd3' in opts else 2), space="PSUM"
            ) as psvp,
            tc.tile_pool(
                name="psd", bufs=(3 if 'psd3' in opts else 2), space="PSUM"
            ) as psdp,
        ):
            eye_sb = cpool.tile([128, 128], F16, tag="eye")
            dneg_sb = cpool.tile([128, 128], F16, tag="dneg")
            nc.sync.dma_start(out=eye_sb[:], in_=eye_h.ap())
            nc.sync.dma_start(out=dneg_sb[:], in_=dneg_h.ap())
            af_sb = cpool.tile([12, BPC * N], F16, tag="af")
            bf_sb = cpool.tile([12, BPC * N], F16, tag="bf")
            nc.sync.dma_start(out=af_sb[:], in_=af.ap())
            nc.sync.dma_start(out=bf_sb[:], in_=bf.ap())

            acc_d2 = accp.tile([128, BPC * NRB], F32, tag="acc_d2")
            acc_u2 = accp.tile([128, BPC * NRB], F32, tag="acc_u2")

            eT_tiles = {}
            for b in range(BPC):
                # ---- normalize + transpose both embedding matrices ----
                eTs = eTp.tile([128, KC, N], BF16, tag=f"eTs{b % 2}")
                eTt = eTp.tile([128, KC, N], BF16, tag=f"eTt{b % 2}")
                eTtn = eTp.tile([128, KC, N], BF16, tag=f"eTtn{b % 2}")
                eT_tiles[b] = (eTs, eTt, eTtn)
                for src, dst in (() if 'no_norm' in opts else ((student, eTs), (teacher, eTt))):
                    raw = rawp.tile([128, NRB, D], F32, tag="raw")
                    if 'abl_noraw' not in opts:
                        src_r = src.ap()[b].rearrange("(r p) d -> p r d", p=128)
                        h = NRB // 2
                        nc.sync.dma_start(out=raw[:, 0:h], in_=src_r[:, 0:h])
                        nc.sync.dma_start(out=raw[:, h:NRB], in_=src_r[:, h:NRB])
                    else:
                        nc.vector.memset(raw[:, 0, 0:4], 1.0)
                    nrm2 = nrmp.tile([128, NRB], F32, tag="nrm2")
                    for rb in range(NRB):
                        sqs = sqscrp.tile([128, D], F32, tag="sqs")
                        if 'sq_dve' in opts:
                            nc.vector.tensor_tensor_reduce(
                                sqs[:], raw[:, rb], raw[:, rb], 1.0, 0.0,
                                op0=OP.mult, op1=OP.add,
                                accum_out=nrm2[:, rb : rb + 1],
                            )
                        else:
                            nc.scalar.activation(
                                sqs[:], raw[:, rb], AF.Square,
                                accum_out=nrm2[:, rb : rb + 1],
                            )
                    rinv = nrmp.tile([128, NRB], F32, tag="rinv")
                    nc.scalar.activation(rinv[:], nrm2[:], AF.Sqrt)
                    nc.vector.tensor_scalar_max(rinv[:], rinv[:], EPS)
                    nc.vector.reciprocal(rinv[:], rinv[:])
                    stage = dramp.tile([N, D], BF16, tag="stage")
                    for rb in range(NRB):
                        eh = ehatp.tile([128, D], BF16, tag="ehat")
                        if 'nrmcopy_act' in opts:
                            nc.scalar.activation(
                                eh[:], raw[:, rb], AF.Copy,
                                scale=rinv[:, rb : rb + 1],
                            )
                        else:
                            nc.vector.tensor_scalar(
                                eh[:], raw[:, rb], rinv[:, rb : rb + 1], None,
                                op0=OP.mult,
                            )
                        nc.sync.dma_start(
                            out=stage[rb * 128 : (rb + 1) * 128, :], in_=eh[:]
                        )
                    for c in range(KC):
                        teng = nc.sync if c % 2 == 0 else nc.scalar
                        teng.dma_start(
                            out=dst[:, c, :],
                            in_=stage[:, c * 128 : (c + 1) * 128],
                            transpose=True,
                        )
                if 'no_norm' not in opts:
                    neng = nc.gpsimd if 'tneg_gpsimd' in opts else nc.vector
                    for c in range(KC):
                        neng.tensor_scalar_mul(eTtn[:, c, :], eTt[:, c, :], -1.0)

                # ---- per row-block: knn threshold + Gram-diff + loss ----
                for rb in range(NRB):
                    rbs = slice(rb * 128, (rb + 1) * 128)
                    # v = -d2 with -inf diagonal
                    skip_v = 'no_vpath' in opts
                    psv = psvp.tile([128, N], F32, tag="psv")
                    for half in ([] if skip_v else range(2)):
                        js = slice(half * 512, (half + 1) * 512)
                        diag_here = (rb // 4) == half
                        nc.tensor.matmul(
                            psv[:, js],
                            af_sb[:, b * N + rb * 128 : b * N + (rb + 1) * 128],
                            bf_sb[:, b * N + half * 512 : b * N + (half + 1) * 512],
                            start=True,
                            stop=True,
                        )
                        if diag_here:
                            nc.tensor.matmul(
                                psv[:, rbs],
                                eye_sb[:],
                                dneg_sb[:],
                                start=False,
                                stop=True,
                                skip_group_check=True,
                            )
                    vsb = vsbp.tile([128, N], F32, tag="vsb")
                    if 'vcopy_dve' in opts:
                        nc.vector.tensor_copy(vsb[:], psv[:])
                    else:
                        nc.scalar.activation(vsb[:], psv[:], AF.Copy)
                    vals8 = smallp.tile([128, 8], F32, tag="vals8")
                    nc.vector.max(vals8[:], vsb[:])

                    # d = cos_s - cos_t accumulated in PSUM
                    psd = psdp.tile([128, N], F32, tag="psd")
                    for half in ([] if 'no_gram' in opts else range(2)):
                        js = slice(half * 512, (half + 1) * 512)
                        for c in range(KC):
                            nc.tensor.matmul(
                                psd[:, js], eTs[:, c, rbs], eTs[:, c, js],
                                start=(c == 0), stop=False,
                            )
                        for c in range(KC):
                            nc.tensor.matmul(
                                psd[:, js], eTtn[:, c, rbs], eTt[:, c, js],
                                start=False, stop=(c == KC - 1),
                            )

                    # dm = (v >= t) * d   (t = 8th largest v in the row)
                    dm = smallp.tile([128, N], BF16, tag="dm")
                    nc.vector.scalar_tensor_tensor(
                        dm[:], vsb[:], vals8[:, 7:8], psd[:],
                        op0=OP.is_ge, op1=OP.mult,
                    )
                    # u = relu(|dm| - beta/2) = relu(dm-0.25*2)... (beta=0.5)
                    ueng = nc.gpsimd if 'u_gpsimd' in opts else nc.vector
                    u1 = smallp.tile([128, N], BF16, tag="u1")
                    ueng.tensor_scalar(
                        u1[:], dm[:], 0.5 * BETA, 0.0, op0=OP.subtract, op1=OP.max
                    )
                    m2 = smallp.tile([128, N], BF16, tag="m2")
                    ueng.tensor_scalar(
                        m2[:], dm[:], 0.5 * BETA, 0.0, op0=OP.add, op1=OP.min
                    )
                    u = smallp.tile([128, N], BF16, tag="u")
                    if 'uonly_gpsimd' in opts:
                        nc.gpsimd.tensor_sub(u[:], u1[:], m2[:])
                    else:
                        nc.vector.tensor_sub(u[:], u1[:], m2[:])
                    # sum dm^2 and u^2 (sl1 = dm^2 - u^2 summed later)
                    col = b * NRB + rb
                    dsq = smallp.tile([128, N], BF16, tag="dsq")
                    if 'dsq_dve' in opts:
                        nc.vector.scalar_tensor_tensor(
                            dsq[:], dm[:], 1.0, dm[:],
                            op0=OP.mult, op1=OP.mult,
                            accum_out=acc_d2[:, col : col + 1],
                        )
                    else:
                        nc.scalar.activation(
                            dsq[:], dm[:], AF.Square,
                            accum_out=acc_d2[:, col : col + 1],
                        )
                    usq = smallp.tile([128, N], BF16, tag="usq")
                    if 'usq_act' in opts:
                        nc.scalar.activation(
                            usq[:], u[:], AF.Square,
                            accum_out=acc_u2[:, col : col + 1],
                        )
                    else:
                        nc.vector.scalar_tensor_tensor(
                            usq[:], u[:], 1.0, u[:],
                            op0=OP.mult, op1=OP.mult,
                            accum_out=acc_u2[:, col : col + 1],
                        )

            s1 = smallp.tile([128, 1], F32, tag="s1")
            s2 = smallp.tile([128, 1], F32, tag="s2")
            nc.vector.reduce_sum(s1[:], acc_d2[:], axis=mybir.AxisListType.X)
            nc.vector.reduce_sum(s2[:], acc_u2[:], axis=mybir.AxisListType.X)
            osb = smallp.tile([128, 1], F32, tag="osb")
            nc.vector.tensor_sub(osb[:], s1[:], s2[:])
            nc.sync.dma_start(out=out.ap(), in_=osb[:])

    nc.compile()
    return nc


def _host_factors(centers_core: np.ndarray) -> tuple[np.ndarray, np.ndarray]:
    """fp16 split-precision rank-12 factors so that
    af[:, i].T @ bf[:, j] ~= -||c_i - c_j||^2 (hi*hi + hi*lo + lo*hi)."""
    x = centers_core[:, :, 0].astype(np.float32)  # [BPC, N]
    y = centers_core[:, :, 1].astype(np.float32)
    sq = x * x + y * y
    ones = np.ones_like(x)
    af = np.stack([-sq, -ones, 2.0 * x, 2.0 * y], axis=1)  # [BPC, 4, N]
    bf = np.stack([ones, sq, x, y], axis=1)                # [BPC, 4, N]
    af = np.ascontiguousarray(af.transpose(1, 0, 2).reshape(4, BPC * N))
    bf = np.ascontiguousarray(bf.transpose(1, 0, 2).reshape(4, BPC * N))
    afh = af.astype(np.float16)
    afl = (af - afh.astype(np.float32)).astype(np.float16)
    bfh = bf.astype(np.float16)
    bfl = (bf - bfh.astype(np.float32)).astype(np.float16)
    af12 = np.ascontiguousarray(np.concatenate([afh, afh, afl], axis=0))
    bf12 = np.ascontiguousarray(np.concatenate([bfh, bfl, bfh], axis=0))
    return af12, bf12


def kernel(student_emb, teacher_emb, centers):
    student_emb = np.asarray(student_emb, dtype=np.float32)
    teacher_emb = np.asarray(teacher_emb, dtype=np.float32)
    centers = np.asarray(centers, dtype=np.float32)

    if "nc" not in _cache:
        _cache["nc"] = _build_nc(("usq_act",))
    nc = _cache["nc"]

    in_maps = []
    for c in range(NCORES):
        lo, hi = c * BPC, (c + 1) * BPC
        af, bf = _host_factors(centers[lo:hi])
        in_maps.append(
            {
                "student": np.ascontiguousarray(student_emb[lo:hi]),
                "teacher": np.ascontiguousarray(teacher_emb[lo:hi]),
                "af": af,
                "bf": bf,
            }
        )

    res = bass_utils.run_bass_kernel_spmd(nc, in_maps, core_ids=list(range(NCORES)))
    total = np.float64(0.0)
    for c in range(NCORES):
        total += np.sum(res.results[c]["out"].astype(np.float64))
    loss = total / float(B * N * 8)
    return np.float32(loss)


# revision 21
# speedup vs baseline: 1.3609x; 1.1633x over previous
"""Trainium2 Bass kernel for LocalRelationDistillLoss.

Full inputs: student_emb [16,1024,768] f32, teacher_emb [16,1024,768] f32,
centers [16,1024,2] f32. Output: scalar f32 loss.

Strategy: data-parallel over batch across 8 NeuronCores (2 batch elements per
core). Per batch element:
  - v = -pairwise_sq_dist(centers) [1024,1024] via a rank-4 augmented matmul
    (factors built on host from centers), with -1e30 added on the diagonal
    (extra identity matmul) so self-matches are excluded.
  - per-row kNN threshold t = 8th-largest of v via the DVE max8 instruction;
    mask = (v >= t) selects exactly the 8 nearest neighbors.
  - cosine similarities via Gram matmul of row-normalized embeddings (bf16).
    Student Gram and negated-teacher Gram accumulate into the same PSUM so
    d = cos_s - cos_t comes out of the PE directly.
  - dm = mask * d in one fused DVE op; smooth-L1(beta=0.5) via
    sl1 = dm^2 - relu(|dm| - 0.5)^2, accumulated with free-dim accum_out.
Per-core output: [128,1] partial sums; host sums and divides.
"""

import os

os.environ.setdefault("MYCRO_LOCAL_CACHE", "1")

import numpy as np
import ml_dtypes

import concourse.bass as bass
import concourse.tile as tile
from concourse import bacc, mybir
from concourse import bass_utils

F32 = mybir.dt.float32
F16 = mybir.dt.float16
BF16 = mybir.dt.bfloat16

B = 16
N = 1024
D = 768
NCORES = 8
BPC = B // NCORES          # batch elements per core
NRB = N // 128             # row blocks
KC = D // 128              # contraction chunks
BETA = 0.5
EPS = 1e-8
NEG_BIG = -1.0e30

_cache = {}


def _build_nc(opts=()):
    opts = set(opts)
    nc = bacc.Bacc("TRN2", target_bir_lowering=False, debug=False)

    student = nc.dram_tensor("student", [BPC, N, D], F32, kind="ExternalInput")
    teacher = nc.dram_tensor("teacher", [BPC, N, D], F32, kind="ExternalInput")
    # augmented fp16 split-precision factors for v = -d2 (hi*hi + hi*lo + lo*hi)
    af = nc.dram_tensor("af", [12, BPC * N], F16, kind="ExternalInput")
    bf = nc.dram_tensor("bf", [12, BPC * N], F16, kind="ExternalInput")
    out = nc.dram_tensor("out", [128, 1], F32, kind="ExternalOutput")

    eye_h = nc.inline_tensor(np.eye(128, dtype=np.float16), "eye128")
    eyebf_h = nc.inline_tensor(
        np.eye(128).astype(ml_dtypes.bfloat16), "eye128bf"
    )
    dneg_h = nc.inline_tensor(
        (-60000.0 * np.eye(128)).astype(np.float16), "diagneg"
    )

    AF = mybir.ActivationFunctionType
    OP = mybir.AluOpType

    with tile.TileContext(nc) as tc:
        with (
            tc.tile_pool(name="const", bufs=1) as cpool,
            tc.tile_pool(name="raw", bufs=(3 if 'raw3' in opts else 2)) as rawp,
            tc.tile_pool(name="ehat", bufs=(8 if 'pehead' in opts else 4)) as ehatp,
            tc.tile_pool(name="sqscr", bufs=2) as sqscrp,
            tc.tile_pool(name="nrm", bufs=4) as nrmp,
            tc.tile_pool(name="eT", bufs=1) as eTp,
            tc.tile_pool(name="vsb", bufs=(3 if 'vsb3' in opts else 2)) as vsbp,
            tc.tile_pool(name="small", bufs=3) as smallp,
            tc.tile_pool(name="accs", bufs=1) as accp,
            tc.tile_pool(name="dram", bufs=2, space="DRAM") as dramp,
            tc.tile_pool(
                name="psv",
                bufs=(1 if ('psd3' in opts or 'pehead' in opts) else 2),
                space="PSUM",
            ) as psvp,
            tc.tile_pool(
                name="psd", bufs=(3 if 'psd3' in opts else 2), space="PSUM"
            ) as psdp,
            tc.tile_pool(name="ptrans", bufs=2, space="PSUM") as ptp,
        ):
            eye_sb = cpool.tile([128, 128], F16, tag="eye")
            dneg_sb = cpool.tile([128, 128], F16, tag="dneg")
            nc.sync.dma_start(out=eye_sb[:], in_=eye_h.ap())
            nc.sync.dma_start(out=dneg_sb[:], in_=dneg_h.ap())
            eyebf_sb = cpool.tile([128, 128], BF16, tag="eyebf")
            nc.sync.dma_start(out=eyebf_sb[:], in_=eyebf_h.ap())
            af_sb = cpool.tile([12, BPC * N], F16, tag="af")
            bf_sb = cpool.tile([12, BPC * N], F16, tag="bf")
            nc.sync.dma_start(out=af_sb[:], in_=af.ap())
            nc.sync.dma_start(out=bf_sb[:], in_=bf.ap())

            acc_d2 = accp.tile([128, BPC * NRB], F32, tag="acc_d2")
            acc_u2 = accp.tile([128, BPC * NRB], F32, tag="acc_u2")

            eT_tiles = {}
            for b in range(BPC):
                # ---- normalize + transpose both embedding matrices ----
                eTs = eTp.tile([128, KC, N], BF16, tag=f"eTs{b % 2}")
                eTt = eTp.tile([128, KC, N], BF16, tag=f"eTt{b % 2}")
                eTtn = eTp.tile([128, KC, N], BF16, tag=f"eTtn{b % 2}")
                eT_tiles[b] = (eTs, eTt, eTtn)
                for src, dst in (() if 'no_norm' in opts else ((student, eTs), (teacher, eTt))):
                    raw = rawp.tile([128, NRB, D], F32, tag="raw")
                    if 'abl_noraw' not in opts:
                        src_r = src.ap()[b].rearrange("(r p) d -> p r d", p=128)
                        h = NRB // 2
                        nc.sync.dma_start(out=raw[:, 0:h], in_=src_r[:, 0:h])
                        nc.sync.dma_start(out=raw[:, h:NRB], in_=src_r[:, h:NRB])
                    else:
                        nc.vector.memset(raw[:, 0, 0:4], 1.0)
                    nrm2 = nrmp.tile([128, NRB], F32, tag="nrm2")
                    for rb in range(NRB):
                        sqs = sqscrp.tile([128, D], F32, tag="sqs")
                        if 'sq_dve' in opts:
                            nc.vector.tensor_tensor_reduce(
                                sqs[:], raw[:, rb], raw[:, rb], 1.0, 0.0,
                                op0=OP.mult, op1=OP.add,
                                accum_out=nrm2[:, rb : rb + 1],
                            )
                        else:
                            nc.scalar.activation(
                                sqs[:], raw[:, rb], AF.Square,
                                accum_out=nrm2[:, rb : rb + 1],
                            )
                    rinv = nrmp.tile([128, NRB], F32, tag="rinv")
                    nc.scalar.activation(rinv[:], nrm2[:], AF.Sqrt)
                    nc.vector.tensor_scalar_max(rinv[:], rinv[:], EPS)
                    nc.vector.reciprocal(rinv[:], rinv[:])
                    pe_head = (
                        ('pehead' in opts and b == 0)
                        and not ('peheadS' in opts and src is teacher)
                    )
                    ehs = []
                    stage = None if pe_head else dramp.tile([N, D], BF16, tag="stage")
                    for rb in range(NRB):
                        eh = ehatp.tile(
                            [128, D], BF16,
                            tag=("ehat_hd" if pe_head else "ehat"),
                        )
                        if 'nrmcopy_act' in opts:
                            nc.scalar.activation(
                                eh[:], raw[:, rb], AF.Copy,
                                scale=rinv[:, rb : rb + 1],
                            )
                        else:
                            nc.vector.tensor_scalar(
                                eh[:], raw[:, rb], rinv[:, rb : rb + 1], None,
                                op0=OP.mult,
                            )
                        if pe_head:
                            ehs.append(eh)
                        else:
                            nc.sync.dma_start(
                                out=stage[rb * 128 : (rb + 1) * 128, :], in_=eh[:]
                            )
                    if pe_head:
                        for c in range(KC):
                            ptc = ptp.tile([128, NRB, 128], BF16, tag="ptc")
                            for rb in range(NRB):
                                nc.tensor.transpose(
                                    ptc[:, rb, :],
                                    ehs[rb][:, c * 128 : (c + 1) * 128],
                                    eyebf_sb[:],
                                )
                            nc.vector.tensor_copy(
                                dst[:, c, :].rearrange("p (r n) -> p r n", r=NRB),
                                ptc[:],
                            )
                    else:
                        for c in range(KC):
                            teng = nc.sync if c % 2 == 0 else nc.scalar
                            teng.dma_start(
                                out=dst[:, c, :],
                                in_=stage[:, c * 128 : (c + 1) * 128],
                                transpose=True,
                            )
                if 'no_norm' not in opts:
                    neng = nc.gpsimd if 'tneg_gpsimd' in opts else nc.vector
                    for c in range(KC):
                        neng.tensor_scalar_mul(eTtn[:, c, :], eTt[:, c, :], -1.0)

                # ---- per row-block: knn threshold + Gram-diff + loss ----
                for rb in range(NRB):
                    rbs = slice(rb * 128, (rb + 1) * 128)
                    # v = -d2 with -inf diagonal
                    skip_v = 'no_vpath' in opts
                    psv = psvp.tile([128, N], F32, tag="psv")
                    for half in ([] if skip_v else range(2)):
                        js = slice(half * 512, (half + 1) * 512)
                        diag_here = (rb // 4) == half
                        nc.tensor.matmul(
                            psv[:, js],
                            af_sb[:, b * N + rb * 128 : b * N + (rb + 1) * 128],
                            bf_sb[:, b * N + half * 512 : b * N + (half + 1) * 512],
                            start=True,
                            stop=True,
                        )
                        if diag_here:
                            nc.tensor.matmul(
                                psv[:, rbs],
                                eye_sb[:],
                                dneg_sb[:],
                                start=False,
                                stop=True,
                                skip_group_check=True,
                            )
                    vsb = vsbp.tile([128, N], F32, tag="vsb")
                    if 'vcopy_dve' in opts:
                        nc.vector.tensor_copy(vsb[:], psv[:])
                    else:
                        nc.scalar.activation(vsb[:], psv[:], AF.Copy)
                    vals8 = smallp.tile([128, 8], F32, tag="vals8")
                    nc.vector.max(vals8[:], vsb[:])

                    # d = cos_s - cos_t accumulated in PSUM
                    psd = psdp.tile([128, N], F32, tag="psd")
                    for half in ([] if 'no_gram' in opts else range(2)):
                        js = slice(half * 512, (half + 1) * 512)
                        for c in range(KC):
                            nc.tensor.matmul(
                                psd[:, js], eTs[:, c, rbs], eTs[:, c, js],
                                start=(c == 0), stop=False,
                            )
                        for c in range(KC):
                            nc.tensor.matmul(
                                psd[:, js], eTtn[:, c, rbs], eTt[:, c, js],
                                start=False, stop=(c == KC - 1),
                            )

                    # dm = (v >= t) * d   (t = 8th largest v in the row)
                    dm = smallp.tile([128, N], BF16, tag="dm")
                    nc.vector.scalar_tensor_tensor(
                        dm[:], vsb[:], vals8[:, 7:8], psd[:],
                        op0=OP.is_ge, op1=OP.mult,
                    )
                    # u = relu(|dm| - beta/2) = relu(dm-0.25*2)... (beta=0.5)
                    ueng = nc.gpsimd if 'u_gpsimd' in opts else nc.vector
                    u1 = smallp.tile([128, N], BF16, tag="u1")
                    ueng.tensor_scalar(
                        u1[:], dm[:], 0.5 * BETA, 0.0, op0=OP.subtract, op1=OP.max
                    )
                    m2 = smallp.tile([128, N], BF16, tag="m2")
                    ueng.tensor_scalar(
                        m2[:], dm[:], 0.5 * BETA, 0.0, op0=OP.add, op1=OP.min
                    )
                    u = smallp.tile([128, N], BF16, tag="u")
                    if 'uonly_gpsimd' in opts:
                        nc.gpsimd.tensor_sub(u[:], u1[:], m2[:])
                    else:
                        nc.vector.tensor_sub(u[:], u1[:], m2[:])
                    # sum dm^2 and u^2 (sl1 = dm^2 - u^2 summed later)
                    col = b * NRB + rb
                    dsq = smallp.tile([128, N], BF16, tag="dsq")
                    if 'dsq_dve' in opts:
                        nc.vector.scalar_tensor_tensor(
                            dsq[:], dm[:], 1.0, dm[:],
                            op0=OP.mult, op1=OP.mult,
                            accum_out=acc_d2[:, col : col + 1],
                        )
                    else:
                        nc.scalar.activation(
                            dsq[:], dm[:], AF.Square,
                            accum_out=acc_d2[:, col : col + 1],
                        )
                    usq = smallp.tile([128, N], BF16, tag="usq")
                    if 'usq_act' in opts:
                        nc.scalar.activation(
                            usq[:], u[:], AF.Square,
                            accum_out=acc_u2[:, col : col + 1],
                        )
                    else:
                        nc.vector.scalar_tensor_tensor(
                            usq[:], u[:], 1.0, u[:],
                            op0=OP.mult, op1=OP.mult,
                            accum_out=acc_u2[:, col : col + 1],
                        )

            s1 = smallp.tile([128, 1], F32, tag="s1")
            s2 = smallp.tile([128, 1], F32, tag="s2")
            nc.vector.reduce_sum(s1[:], acc_d2[:], axis=mybir.AxisListType.X)
            nc.vector.reduce_sum(s2[:], acc_u2[:], axis=mybir.AxisListType.X)
            osb = smallp.tile([128, 1], F32, tag="osb")
            nc.vector.tensor_sub(osb[:], s1[:], s2[:])
            nc.sync.dma_start(out=out.ap(), in_=osb[:])

    nc.compile()
    return nc


def _host_factors(centers_core: np.ndarray) -> tuple[np.ndarray, np.ndarray]:
    """fp16 split-precision rank-12 factors so that
    af[:, i].T @ bf[:, j] ~= -||c_i - c_j||^2 (hi*hi + hi*lo + lo*hi)."""
    x = centers_core[:, :, 0].astype(np.float32)  # [BPC, N]
    y = centers_core[:, :, 1].astype(np.float32)
    sq = x * x + y * y
    ones = np.ones_like(x)
    af = np.stack([-sq, -ones, 2.0 * x, 2.0 * y], axis=1)  # [BPC, 4, N]
    bf = np.stack([ones, sq, x, y], axis=1)                # [BPC, 4, N]
    af = np.ascontiguousarray(af.transpose(1, 0, 2).reshape(4, BPC * N))
    bf = np.ascontiguousarray(bf.transpose(1, 0, 2).reshape(4, BPC * N))
    afh = af.astype(np.float16)
    afl = (af - afh.astype(np.float32)).astype(np.float16)
    bfh = bf.astype(np.float16)
    bfl = (bf - bfh.astype(np.float32)).astype(np.float16)
    af12 = np.ascontiguousarray(np.concatenate([afh, afh, afl], axis=0))
    bf12 = np.ascontiguousarray(np.concatenate([bfh, bfl, bfh], axis=0))
    return af12, bf12


def kernel(student_emb, teacher_emb, centers):
    student_emb = np.asarray(student_emb, dtype=np.float32)
    teacher_emb = np.asarray(teacher_emb, dtype=np.float32)
    centers = np.asarray(centers, dtype=np.float32)

    if "nc" not in _cache:
        _cache["nc"] = _build_nc(("usq_act", "pehead"))
    nc = _cache["nc"]

    in_maps = []
    for c in range(NCORES):
        lo, hi = c * BPC, (c + 1) * BPC
        af, bf = _host_factors(centers[lo:hi])
        in_maps.append(
            {
                "student": np.ascontiguousarray(student_emb[lo:hi]),
                "teacher": np.ascontiguousarray(teacher_emb[lo:hi]),
                "af": af,
                "bf": bf,
            }
        )

    res = bass_utils.run_bass_kernel_spmd(nc, in_maps, core_ids=list(range(NCORES)))
    total = np.float64(0.0)
    for c in range(NCORES):
        total += np.sum(res.results[c]["out"].astype(np.float64))
    loss = total / float(B * N * 8)
    return np.float32(loss)
